# revision 38
# baseline (speedup 1.0000x reference)
"""Trainium2 Bass kernel: nn_MultiHeadCrossAttention (B=4, S=1024, H=1024, 16 heads).

Sharding: 8 cores = (batch b in 0..3) x (head-group g in 0..1, 8 heads each).
Per core: q/k/v projections for its head group on its batch, flash-style
attention in scores-transposed layout (softmax along the PSUM partition axis
via an augmented ones-column in the v matmul), and a partial out-projection.
Host sums the two per-batch partials and adds the output bias.

The bilinear span bias of the reference is constant along the softmax key
axis, so it cancels exactly in softmax and is not computed.

All matmul operands are fp16 (1 cycle/row on the PE vs ~1.4 measured for
fp32r), fp32 PSUM accumulate. Host converts inputs/weights to fp16; measured
rel-l2 vs fp64 reference ~7e-4 (gate is 2e-2).

Softmax normalization: 64x-replicated ones-columns in the augmented v make
the ctx matmuls deposit per-query exp-sums on half the PSUM partitions.
The full PSUM tiles are copied to fp16 SBUF immediately (freeing the
accumulator banks for the next head pair), then the sums rows are
DMA-repartitioned to [128,4] for a cheap exact DVE reciprocal (which costs
~6.3 cycles per FREE element), bounced through DRAM, and broadcast-read
back with a 0-stride partition AP for the final normalize multiply.
"""
import os
import sys
import types

sys.path.insert(0, "/opt/trn_rl_repo")

# Optional NTFF profile hook shim (axon images lack antenv.axon_hooks).
# Only used when tracing is requested; harmless otherwise.
if "antenv.axon_hooks" not in sys.modules:
    try:
        import trn_agent_boot.trn_boot as _tb

        _m = types.ModuleType("antenv.axon_hooks")
        _m.get_axon_ntff_profile_hook = (
            lambda: _tb._ntff_profile_via_ctypes("/opt/axon/libaxon_pjrt.so")
        )
        _m.set_axon_ntff_profile_hook = lambda h: None
        sys.modules["antenv.axon_hooks"] = _m
    except Exception:
        pass

import numpy as np

import concourse.bass as bass
import concourse.mybir as mybir
import concourse.tile as tile
from concourse import bacc
from concourse.bass_utils import run_bass_kernel_spmd

F32 = mybir.dt.float32
F16 = mybir.dt.float16
AF = mybir.ActivationFunctionType

B, S, H = 4, 1024, 1024
NHEADS = 16
HD = 64
G = 2                  # head groups (cores per batch)
NH = NHEADS // G       # 8 heads per core
F = NH * HD            # 512 per-core qkv features
HC = H // 128          # 8 contraction chunks for projections
KT = S // 128          # 8 key tiles
ST = S // 128          # 8 seq tiles
FC = F // 128          # 4 feature chunks
NQ = S // 512          # 2 query halves
SCALE = float(HD) ** -0.5

# Augmented v region per head pair, width 256 (two 128-wide blocks):
#   even block cols 0..127:   v_even (0..63)  | ones (64..127)
#   odd  block cols 128..255: ones (128..191) | v_odd (192..255)
# even ctx matmul (cols 0:128):   out p0-63 = ctx_e, p64-127 = sums_e (x64)
# odd  ctx matmul (cols 128:256): out p0-63 = sums_o (x64), p64-127 = ctx_o
# The 64x-replicated ones columns make the PE broadcast the softmax
# denominator across partitions for free, so normalization is two
# full-width DVE reciprocals + two multiplies.
VREG = 256

_CACHE: dict = {}


def _build_nc():
    nc = bacc.Bacc("TRN2", target_bir_lowering=False, debug=False)

    xT = nc.dram_tensor("xT", [H, S], F16, kind="ExternalInput")    # aspect[b].T
    yT = nc.dram_tensor("yT", [H, S], F16, kind="ExternalInput")    # opinion[b].T
    wqT = nc.dram_tensor("wqT", [H, F], F16, kind="ExternalInput")  # Wq[g].T
    wkT = nc.dram_tensor("wkT", [H, F], F16, kind="ExternalInput")
    wvT = nc.dram_tensor("wvT", [H, F], F16, kind="ExternalInput")
    woT = nc.dram_tensor("woT", [F, H], F16, kind="ExternalInput")  # Wo[:, g].T
    bqv = nc.dram_tensor("bqv", [F], F32, kind="ExternalInput")
    bkv = nc.dram_tensor("bkv", [F], F32, kind="ExternalInput")
    ebias = nc.dram_tensor("ebias", [S], F32, kind="ExternalInput")  # mask bias per key
    out = nc.dram_tensor("out", [S, H], F32, kind="ExternalOutput")
    # DRAM bounce for the softmax reciprocals (DRAM APs allow the 0-stride
    # partition-broadcast read that SBUF APs reject).
    rsc = nc.dram_tensor("rsc", [NH // 2 * NQ, 2, 512], F32)

    with tile.TileContext(nc) as tc:
        const = tc.alloc_tile_pool(name="const", bufs=1)
        persist = tc.alloc_tile_pool(name="persist", bufs=1)

        bq_sb = const.tile([128, FC], F32, name="bq_sb")
        nc.sync.dma_start(out=bq_sb, in_=bqv.rearrange("(c p) -> p c", p=128))
        bk_sb = const.tile([128, FC], F32, name="bk_sb")
        nc.sync.dma_start(out=bk_sb, in_=bkv.rearrange("(c p) -> p c", p=128))
        eb_sb = const.tile([128, KT], F32, name="eb_sb")
        nc.sync.dma_start(out=eb_sb, in_=ebias.rearrange("(c p) -> p c", p=128))
        wo_sb = const.tile([128, FC, H], F16, name="wo_sb")

        qT_sb = persist.tile([128, FC, S], F16, name="qT_sb")
        # kT as one tile per feature chunk: attn hp0 then depends only on
        # k-proj fc0, so k-proj fc1-3 become ready filler work for the
        # exp-paced attention kt loops.
        kT_fcs = [persist.tile([128, S], F16, name=f"kT_fc{fc}")
                  for fc in range(FC)]
        v_sb = persist.tile([128, KT, NH // 2, VREG], F16, name="v_sb")
        # ctx as one tile per (query half, feature chunk): deps are per-tile,
        # so this lets out-proj's fc0-2 accumulation matmuls run as soon as
        # those head pairs' normalizes land, instead of waiting for the last
        # head pair (measured 9.8us PE gap before the final out-proj burst).
        ctx_sbs = [[persist.tile([128, 512], F16, name=f"ctx_sb{nq}_{fc}")
                    for fc in range(FC)] for nq in range(NQ)]

        nc.vector.memset(
            v_sb.rearrange("p a b c -> p (a b) c")[:, :, 64:192], 1.0)

        # ---------------- projections (inputs released afterwards) ----------
        psum = tc.alloc_tile_pool(name="psum", bufs=1, space="PSUM")
        pjsb = tc.alloc_tile_pool(name="pjsb", bufs=1)
        # wv/yt as one tile PER hc chunk: SBUF dependencies are per-tile, so
        # a single big tile would make the first v-proj matmul wait for every
        # chunk DMA (measured 14.5us); per-chunk tiles unblock it at ~1.5us.
        wv_sbs = [pjsb.tile([128, F], F16, name=f"wv_sb{hc}") for hc in range(HC)]
        yt_sbs = [pjsb.tile([128, S], F16, name=f"yt_sb{hc}") for hc in range(HC)]
        wk_sb = pjsb.tile([128, HC, F], F16, name="wk_sb")
        wq_sb = pjsb.tile([128, HC, F], F16, name="wq_sb")
        xt_sb = pjsb.tile([128, HC, S], F16, name="xt_sb")

        # Input loads split across two engine queues (each queued DMA costs
        # ~0.6-0.8us of queue occupancy, so one queue serializes startup):
        # sync gets wv/wk/xt, the scalar queue (idle until the first exp)
        # gets yt/wq/wo. Issue order matches compute order.
        for hc in range(HC):
            nc.sync.dma_start(
                out=wv_sbs[hc],
                in_=wvT.rearrange("(c p) f -> p c f", p=128)[:, hc, :])
            nc.scalar.dma_start(
                out=yt_sbs[hc],
                in_=yT.rearrange("(c p) s -> p c s", p=128)[:, hc, :])
        nc.sync.dma_start(out=wk_sb,
                          in_=wkT.rearrange("(c p) f -> p c f", p=128))
        nc.scalar.dma_start(out=wq_sb,
                            in_=wqT.rearrange("(c p) f -> p c f", p=128))
        for nq in range(NQ):
            nc.sync.dma_start(
                out=xt_sb[:, :, nq * 512:(nq + 1) * 512],
                in_=xT.rearrange("(c p) s -> p c s", p=128)[
                    :, :, nq * 512:(nq + 1) * 512])
        nc.scalar.dma_start(out=wo_sb,
                            in_=woT.rearrange("(c p) h -> p c h", p=128))

        # v = opinion @ Wv.T : accumulate [s,128]x[128,F] over hc
        for st in range(ST):
            ps = psum.tile([128, F], F32, name="vps", tag="pp", bufs=4)
            for hc in range(HC):
                nc.tensor.matmul(
                    ps,
                    yt_sbs[hc][:, st * 128:(st + 1) * 128],
                    wv_sbs[hc],
                    start=(hc == 0), stop=(hc == HC - 1),
                )
            # scatter per head pair into the augmented v regions
            pv = ps.rearrange("p (hp e d) -> p hp e d", hp=NH // 2, e=2)
            nc.vector.tensor_copy(v_sb[:, st, :, 0:64], pv[:, :, 0, :])
            nc.vector.tensor_copy(v_sb[:, st, :, 192:256], pv[:, :, 1, :])

        def proj_chunk(src_ap, w_sb, b_sb, dst_ap, fc, nq):
            ps = psum.tile([128, 512], F32, name="qkps", tag="pp", bufs=4)
            for hc in range(HC):
                nc.tensor.matmul(
                    ps,
                    w_sb[:, hc, fc * 128:(fc + 1) * 128],
                    src_ap(hc, nq * 512, (nq + 1) * 512),
                    start=(hc == 0), stop=(hc == HC - 1),
                )
            nc.vector.tensor_scalar_add(dst_ap, ps, b_sb[:, fc:fc + 1])

        yt_ap = lambda hc, lo, hi: yt_sbs[hc][:, lo:hi]
        xt_ap = lambda hc, lo, hi: xt_sb[:, hc, lo:hi]

        def k_chunk(fc):
            for nq in range(NQ):
                proj_chunk(yt_ap, wk_sb, bk_sb,
                           kT_fcs[fc][:, nq * 512:(nq + 1) * 512], fc, nq)

        def q_chunk(fc, nq):
            proj_chunk(xt_ap, wq_sb, bq_sb,
                       qT_sb[:, fc, nq * 512:(nq + 1) * 512], fc, nq)

        # k fc0 + q(0,0) are all attn (nq0, hp0) needs; the remaining k and
        # q chunks are interleaved between attention head pairs below as
        # ready filler for the exp-paced kt loops.
        k_chunk(0)
        q_chunk(0, 0)

        # ---------------- attention + out-projection ----------------
        exps = tc.alloc_tile_pool(name="exps", bufs=4)
        outsb = tc.alloc_tile_pool(name="outsb", bufs=3)
        smallp = tc.alloc_tile_pool(name="smallp", bufs=2)
        ctxup = tc.alloc_tile_pool(name="ctxup", bufs=3)

        def attn_hp(nq, hp):
                fc = hp
                cps_e = psum.tile([128, 512], F32, name="cps_e", tag="pp", bufs=4)
                cps_o = psum.tile([128, 512], F32, name="cps_o", tag="pp", bufs=4)
                for kt in range(KT):
                    sps = psum.tile([128, 2, 512], F32, name="sps", tag="sps", bufs=2)
                    for e in range(2):
                        p0 = 64 * e
                        # scoresT[k, q] = k_h . q_h over hd=64
                        nc.tensor.matmul(
                            sps[:, e, :],
                            kT_fcs[fc][p0:p0 + 64, kt * 128:(kt + 1) * 128],
                            qT_sb[p0:p0 + 64, fc, nq * 512:(nq + 1) * 512],
                            start=True, stop=True,
                        )
                    ex = exps.tile([128, 2, 512], F16, name="ex", tag="ex")
                    nc.scalar.activation(
                        ex, sps, AF.Exp,
                        bias=eb_sb[:, kt:kt + 1], scale=SCALE,
                    )
                    nc.tensor.matmul(
                        cps_e,
                        v_sb[:, kt, hp, 0:128],
                        ex[:, 0, :],
                        start=(kt == 0), stop=(kt == KT - 1),
                    )
                    nc.tensor.matmul(
                        cps_o,
                        v_sb[:, kt, hp, 128:256],
                        ex[:, 1, :],
                        start=(kt == 0), stop=(kt == KT - 1),
                    )
                # normalize: ctx * (1/sums). First copy the full PSUM tiles
                # (ctx + replicated sums rows) to fp16 SBUF -- this frees the
                # accumulator banks ~1.3us after the kt loop so the next head
                # pair never stalls on the PSUM ring. The exact DVE
                # reciprocal costs ~6.3 cycles per FREE element, so the 512
                # sums are DMA-repartitioned to [128,4] (free=4, ~200ns),
                # reciprocal'd, bounced through DRAM, and broadcast-read back
                # with a 0-stride partition AP. All proven-on-HW pieces.
                ctxu = ctxup.tile([128, 2, 512], F16, name="ctxu", tag="ctxu")
                nc.vector.tensor_copy(ctxu[:, 0, :], cps_e)
                nc.vector.tensor_copy(ctxu[:, 1, :], cps_o)
                # alternate the norm DMA chain between the gpsimd and sync
                # queues per head pair: a single queue builds up a backlog
                # whose end-of-kernel drain (~5us) blocks the final matmuls.
                dq = nc.gpsimd if hp % 2 == 0 else nc.sync
                sp = smallp.tile([128, 8], F16, name="sp", tag="sp")
                dq.dma_start(out=sp[:, 0:4], in_=ctxu[64:65, 0, :])
                dq.dma_start(out=sp[:, 4:8], in_=ctxu[0:1, 1, :])
                rp = smallp.tile([128, 8], F32, name="rp", tag="rp")
                nc.vector.reciprocal(out=rp, in_=sp)
                it = nq * (NH // 2) + hp
                dq.dma_start(out=rsc[it, 0, :], in_=rp[:, 0:4])
                dq.dma_start(out=rsc[it, 1, :], in_=rp[:, 4:8])
                rt = smallp.tile([128, 512], F32, name="rt", tag="rt")
                for e in range(2):
                    src = rsc[it, e, :]
                    dq.dma_start(
                        out=rt[64 * e:64 * e + 64, :],
                        in_=bass.AP(tensor=src.tensor, offset=src.offset,
                                    ap=[[0, 64]] + list(src.ap)))
                nc.vector.tensor_mul(
                    ctx_sbs[nq][fc][0:64, :], ctxu[0:64, 0, :], rt[0:64, :])
                nc.vector.tensor_mul(
                    ctx_sbs[nq][fc][64:128, :], ctxu[64:128, 1, :],
                    rt[64:128, :])

        def outproj_st(st):
                for no in range(NQ):
                    ps = psum.tile([128, 512], F32, name="ops",
                                   tag="pp", bufs=4)
                    for fc2 in range(FC):
                        nc.tensor.matmul(
                            ps,
                            ctx_sbs[st // 4][fc2][
                                :, (st % 4) * 128:(st % 4 + 1) * 128],
                            wo_sb[:, fc2, no * 512:(no + 1) * 512],
                            start=(fc2 == 0), stop=(fc2 == FC - 1),
                        )
                    ot = outsb.tile([128, 512], F32, name="ot", tag="ot")
                    nc.vector.tensor_copy(ot, ps)
                    # sync queue: idle after the input loads, and keeping
                    # the gpsimd queue short avoids a long final drain.
                    nc.sync.dma_start(
                        out=out[st * 128:(st + 1) * 128,
                                no * 512:(no + 1) * 512],
                        in_=ot)

        # Interleave: after each attention head pair, emit the k/q projection
        # chunks the NEXT head pair needs. They are ready immediately, so the
        # scheduler pulls them into the current head pair's exp-paced PE gaps
        # (~1.9us of PE idle per head pair otherwise).
        for hp in range(NH // 2):
            attn_hp(0, hp)
            if hp < 3:
                k_chunk(hp + 1)
                q_chunk(hp + 1, 0)
            else:
                q_chunk(0, 1)
        # out-proj nq0 s-tiles are emitted interleaved between attn nq1 head
        # pairs: PSUM ring slots are allocated in emission order, so emitting
        # them after all of attn nq1 would also force their execution after
        # it -- interleaved, they fill attn nq1's exp-paced PE gaps.
        for hp in range(NH // 2):
            attn_hp(1, hp)
            if hp < 3:
                q_chunk(hp + 1, 1)
            outproj_st(hp)
        for st in range(4, 8):
            outproj_st(st)

        ctxup.release()
        smallp.release()
        outsb.release()
        exps.release()
        pjsb.release()
        psum.release()
        persist.release()
        const.release()

    nc.finalize()
    return nc


def get_nc():
    if "nc" not in _CACHE:
        _CACHE["nc"] = _build_nc()
    return _CACHE["nc"]


def make_in_maps(aspect_hidden, opinion_hidden, attention_mask,
                 Wq, bq, Wk, bk, Wv, bv, Wo, bo):
    asp = np.asarray(aspect_hidden, np.float32)
    opi = np.asarray(opinion_hidden, np.float32)
    mask = np.asarray(attention_mask)
    in_maps = []
    xTs = [np.ascontiguousarray(asp[b].T.astype(np.float16)) for b in range(B)]
    yTs = [np.ascontiguousarray(opi[b].T.astype(np.float16)) for b in range(B)]
    ebs = [np.where(mask[b] == 0, np.float32(-1e30), np.float32(0.0)).astype(np.float32)
           for b in range(B)]
    wqTs = [np.ascontiguousarray(Wq[g * F:(g + 1) * F, :].T.astype(np.float16))
            for g in range(G)]
    wkTs = [np.ascontiguousarray(Wk[g * F:(g + 1) * F, :].T.astype(np.float16))
            for g in range(G)]
    wvTs = [np.ascontiguousarray(Wv[g * F:(g + 1) * F, :].T.astype(np.float16))
            for g in range(G)]
    woTs = [np.ascontiguousarray(Wo[:, g * F:(g + 1) * F].T.astype(np.float16))
            for g in range(G)]
    bqs = [np.ascontiguousarray(bq[g * F:(g + 1) * F]) for g in range(G)]
    bks = [np.ascontiguousarray(bk[g * F:(g + 1) * F]) for g in range(G)]
    for c in range(8):
        b, g = c // G, c % G
        in_maps.append({
            "xT": xTs[b], "yT": yTs[b],
            "wqT": wqTs[g], "wkT": wkTs[g], "wvT": wvTs[g], "woT": woTs[g],
            "bqv": bqs[g], "bkv": bks[g], "ebias": ebs[b],
        })
    return in_maps


def kernel(aspect_hidden, opinion_hidden, attention_mask,
           Wq, bq, Wk, bk, Wv, bv, Wo, bo, Wbil, bbil):
    Wq = np.asarray(Wq, np.float32); bq = np.asarray(bq, np.float32)
    Wk = np.asarray(Wk, np.float32); bk = np.asarray(bk, np.float32)
    Wv = np.asarray(Wv, np.float32); bv = np.asarray(bv, np.float32)
    Wo = np.asarray(Wo, np.float32); bo = np.asarray(bo, np.float32)

    nc = get_nc()
    in_maps = make_in_maps(aspect_hidden, opinion_hidden, attention_mask,
                           Wq, bq, Wk, bk, Wv, bv, Wo, bo)
    trace = bool(int(os.environ.get("KERNEL_TRACE", "0")))
    res = run_bass_kernel_spmd(nc, in_maps, core_ids=list(range(8)), trace=trace)
    _CACHE["last_results"] = res

    # v-bias folds into a constant output offset: softmax rows sum to 1, so
    # ctx picks up +bv exactly, and out picks up +Wo @ bv.
    bo_eff = (bo.astype(np.float64) + Wo.astype(np.float64) @ bv.astype(np.float64))
    outs = np.empty((B, S, H), np.float32)
    for b in range(B):
        acc = (res.results[G * b]["out"].astype(np.float64)
               + res.results[G * b + 1]["out"].astype(np.float64) + bo_eff)
        outs[b] = acc.astype(np.float32)
    return outs
